# revision 20
# baseline (speedup 1.0000x reference)
"""Causal multi-head attention (RoPE) Trainium2 Bass kernel.

Problem: x[2,2048,1024] @ Wq/Wk/Wv -> 16 heads of causal attention with
interleaved-pair RoPE -> @ Wo.  Sharded over 8 NeuronCores as
(batch x head-group): core c handles batch c//4 and heads [4*(c%4), 4*(c%4)+4).
Each core computes a partial y^T = (attn_out_heads @ Wo[rows]) for its head
group; the host sums the 4 partials per batch and transposes back.

On-device layout is "transposed" throughout: x^T, Q^T, K^T live as
[dims, seq] so every matmul contracts over the partition axis.  Scores are
computed transposed (S^T[kv, q]) so the PV matmul needs no transposes; the
softmax denominator comes from an extra ones-column appended to V.  The
softmax runs without max-subtraction (scores are O(10), exp cannot overflow).

All matmul/DVE operands are bf16 (PSUM accumulation stays fp32): full-rate
PE, fast weight load, half the DMA bytes, 2-4x DVE modes.  The causal mask
is applied as a 0/1 bf16 multiply on exp(scores) (exp of unmasked upper
triangle is finite garbage that the multiply kills).  Emission is software-
pipelined: the projections for chunk qc+1 and the Wo-projection for chunk
qc-1 are interleaved between attention groups of chunk qc, so the PE has
work while the scalar engine computes exp.

RoPE: rope(v) = v*cos + pairswap(v)*sin', where pairswap is a 128x128
permutation matmul and sin' carries the alternating signs.  The reciprocal
of the softmax denominator is computed after spreading the 512 sums across
128 partitions by SBUF-to-SBUF DMA (a [1,512] reciprocal runs on a single
DVE lane and would cost ~4us).
"""

import numpy as np
import ml_dtypes

import concourse.bass as bass
import concourse.mybir as mybir
from concourse import bacc
from concourse.tile import TileContext
from concourse.bass_utils import run_bass_kernel_spmd

P = 128          # partitions
S = 2048         # sequence length
D = 1024         # model dim
DK = 64          # head dim
HPC = 4          # heads per core
DH = HPC * DK    # qkv dims per core (256)
KO = D // P      # 8 contraction slices
QCW = 512        # q chunk width
NQC = S // QCW   # 4 q chunks
NST = S // P     # 16 kv tiles
NCORES = 8

F32 = mybir.dt.float32
BF = mybir.dt.bfloat16
BF_NP = ml_dtypes.bfloat16
EXP = mybir.ActivationFunctionType.Exp
MUL = mybir.AluOpType.mult
ADD = mybir.AluOpType.add


def build_nc():
    nc = bacc.Bacc()

    xt = nc.dram_tensor("xt", [D, S], BF, kind="ExternalInput")
    # weights arrive host-pre-arranged so loads are plain contiguous DMAs:
    # wq[p, ko, m] = Wq[ko*128 + p, m];  wo[p, kb, n] = Wo[kb*128 + p, n]
    wq = nc.dram_tensor("wq", [P, KO, DH], BF, kind="ExternalInput")
    wk = nc.dram_tensor("wk", [P, KO, DH], BF, kind="ExternalInput")
    wv = nc.dram_tensor("wv", [P, KO, DH], BF, kind="ExternalInput")
    wo = nc.dram_tensor("wo", [P, 2, D], BF, kind="ExternalInput")
    cosd = nc.dram_tensor("cosd", [P, S], BF, kind="ExternalInput")
    sind = nc.dram_tensor("sind", [P, S], BF, kind="ExternalInput")
    pswap = nc.dram_tensor("pswap", [P, P], BF, kind="ExternalInput")
    # m01d[p, c] = 1 if c >= p else 0 (keep lower triangle incl. diagonal)
    m01d = nc.dram_tensor("m01d", [P, P], BF, kind="ExternalInput")
    yt = nc.dram_tensor("yt", [D, S], BF, kind="ExternalOutput")

    with TileContext(nc) as tc:
        with (
            tc.tile_pool(name="const", bufs=1) as cp,
            tc.tile_pool(name="qt", bufs=2) as qtp,
            tc.tile_pool(name="ot", bufs=2) as otp,
            tc.tile_pool(name="exps", bufs=3) as expp,
            tc.tile_pool(name="raw", bufs=3) as rawp,
            tc.tile_pool(name="tmp", bufs=3) as tmpp,
            tc.tile_pool(name="rcp", bufs=2) as rcpp,
            tc.tile_pool(name="rbp", bufs=2) as rbpp,
            tc.tile_pool(name="ysb", bufs=3) as ysbp,
            tc.tile_pool(name="psA", bufs=2, space="PSUM") as psA,
            tc.tile_pool(name="psB", bufs=2, space="PSUM") as psB,
            tc.tile_pool(name="psC", bufs=2, space="PSUM") as psC,
        ):
            # ---- constant loads ----
            # gpsimd queue: weights (wk first: it gates the first matmuls);
            # sync/scalar queues: x slices in parallel.
            wk_sb = cp.tile([P, KO, DH], BF, tag="wk")
            nc.gpsimd.dma_start(wk_sb[:, 0:4, :], wk[:, 0:4, :])
            sw_sb = cp.tile([P, P], BF, tag="pswap")
            nc.gpsimd.dma_start(sw_sb[:], pswap[:])
            nc.gpsimd.dma_start(wk_sb[:, 4:KO, :], wk[:, 4:KO, :])
            wv_sb = cp.tile([P, KO, DH], BF, tag="wv")
            nc.gpsimd.dma_start(wv_sb[:], wv[:])
            wq_sb = cp.tile([P, KO, DH], BF, tag="wq")
            nc.gpsimd.dma_start(wq_sb[:], wq[:])
            m01_sb = cp.tile([P, P], BF, tag="m01")
            nc.gpsimd.dma_start(m01_sb[:], m01d[:])
            wo_sb = cp.tile([P, 2, D], BF, tag="wo")
            nc.gpsimd.dma_start(wo_sb[:], wo[:])
            # x arrives in column chunks: the 512 columns chunk 0 needs land
            # before the full rows, so the first projections start early.
            xts = [cp.tile([P, S], BF, tag=f"xt{ko}", name=f"xt{ko}") for ko in range(KO)]
            for ko in range(KO):
                eng = nc.sync if ko % 2 == 0 else nc.scalar
                eng.dma_start(xts[ko][:, 0:QCW], xt[P * ko : P * (ko + 1), 0:QCW])
            cos_sb = cp.tile([P, S], BF, tag="cos")
            nc.sync.dma_start(cos_sb[:, 0:QCW], cosd[:, 0:QCW])
            sin_sb = cp.tile([P, S], BF, tag="sin")
            nc.scalar.dma_start(sin_sb[:, 0:QCW], sind[:, 0:QCW])
            for ko in range(KO):
                eng = nc.sync if ko % 2 == 0 else nc.scalar
                eng.dma_start(xts[ko][:, QCW:S], xt[P * ko : P * (ko + 1), QCW:S])
            nc.sync.dma_start(cos_sb[:, QCW:S], cosd[:, QCW:S])
            nc.scalar.dma_start(sin_sb[:, QCW:S], sind[:, QCW:S])

            # V in [s-rows, dims] layout with a ones column per head:
            # [V(64) | 1] -> PV out rows 0..63 = O^T, row 64 = sum(exp)
            v_sb = cp.tile([P, NST, HPC, DK + 1], BF, tag="v")
            nc.vector.memset(v_sb[:, :, :, DK], 1.0)

            kt_sb = cp.tile([P, 2, S], BF, tag="kt")

            # Warm-up: ~3.4us of back-to-back dummy matmuls on the first
            # tiles to land flips the PE clock-gate to 2.4GHz while the
            # remaining input DMAs are still in flight.
            for _ in range(8):
                wu = psC.tile([P, QCW], F32, tag="proj")
                nc.tensor.matmul(
                    wu[:], sw_sb[:], wk_sb[:, 0:2, :], start=True, stop=True
                )

            def proj_rope(w_sb, dst, pb, qc):
                """dst <- rope( (x @ W)^T )[128 dims block pb, 512 q cols qc]."""
                qs = slice(QCW * qc, QCW * (qc + 1))
                ps = psC.tile([P, QCW], F32, tag="proj")
                for ko in range(KO):
                    nc.tensor.matmul(
                        ps[:],
                        w_sb[:, ko, P * pb : P * (pb + 1)],
                        xts[ko][:, qs],
                        start=(ko == 0),
                        stop=(ko == KO - 1),
                    )
                raw = rawp.tile([P, QCW], BF, tag="raw")
                nc.vector.tensor_copy(raw[:], ps[:])
                swp = psC.tile([P, QCW], F32, tag="proj")
                nc.tensor.matmul(swp[:], sw_sb[:], raw[:], start=True, stop=True)
                nc.vector.tensor_tensor(dst, raw[:], cos_sb[:, qs], MUL)
                tmp = tmpp.tile([P, QCW], BF, tag="tmp")
                nc.vector.tensor_tensor(tmp[:], swp[:], sin_sb[:, qs], MUL)
                nc.vector.tensor_tensor(dst, dst, tmp[:], ADD)

            def v_proj(st):
                ps = psC.tile([P, QCW], F32, tag="proj")
                for ko in range(KO):
                    nc.tensor.matmul(
                        ps[:, :DH],
                        xts[ko][:, P * st : P * (st + 1)],
                        wv_sb[:, ko, :],
                        start=(ko == 0),
                        stop=(ko == KO - 1),
                    )
                vv = ps[:, :DH].rearrange("p (h d) -> p h d", h=HPC)
                nc.vector.tensor_copy(v_sb[:, st, :, 0:DK], vv)

            def out_proj(qc, ot_t, mt):
                """yt[128 rows mt, q chunk qc] = Wo^T[mt] @ O(qc)."""
                qs = slice(QCW * qc, QCW * (qc + 1))
                yp = psC.tile([P, QCW], F32, tag="proj")
                for kb in range(2):
                    nc.tensor.matmul(
                        yp[:],
                        wo_sb[:, kb, P * mt : P * (mt + 1)],
                        ot_t[:, kb, :],
                        start=(kb == 0),
                        stop=(kb == 1),
                    )
                ys = ysbp.tile([P, QCW], BF, tag="ys")
                nc.vector.tensor_copy(ys[:], yp[:])
                nc.sync.dma_start(yt[P * mt : P * (mt + 1), qs], ys[:])

            qts = {}
            ots = {}

            def q_proj(qc, pb):
                if qc not in qts:
                    qts[qc] = qtp.tile([P, 2, QCW], BF, tag="qt", name=f"qt{qc}")
                proj_rope(wq_sb, qts[qc][:, pb, :], pb, qc)

            # ---- prologue: projections for qc 0 ----
            for pb in range(2):
                proj_rope(wk_sb, kt_sb[:, pb, 0:QCW], pb, 0)
            for st in range(4):
                v_proj(st)
            for pb in range(2):
                q_proj(0, pb)

            # ---- software-pipelined main loop ----
            for qc in range(NQC):
                qt_t = qts[qc]

                bg = []  # units interleaved into this qc's attention groups
                if qc >= 1:
                    for mt in range(KO):
                        bg.append(("o", qc - 1, mt))
                if qc + 1 < NQC:
                    for pb in range(2):
                        bg.append(("k", qc + 1, pb))
                    for st in range(4 * (qc + 1), 4 * (qc + 1) + 4):
                        bg.append(("v", st, 0))
                    for pb in range(2):
                        bg.append(("q", qc + 1, pb))

                def emit(u):
                    kind, a, b = u
                    if kind == "o":
                        out_proj(a, ots[a], b)
                    elif kind == "k":
                        proj_rope(wk_sb, kt_sb[:, b, QCW * a : QCW * (a + 1)], b, a)
                    elif kind == "v":
                        v_proj(a)
                    else:
                        q_proj(a, b)

                ot_t = otp.tile([P, 2, QCW], BF, tag="ot")
                ots[qc] = ot_t
                nst = 4 * qc + 4          # kv tiles for this chunk (causal)
                ngr = nst // 2            # processed in pairs
                total_groups = HPC * ngr
                done = 0
                bg_i = 0
                # The normalize multiply of head h is emitted during head
                # h+1's groups so the DVE queue never head-of-line blocks
                # on the partition broadcast.
                pending_mul = [None]

                def flush_mul():
                    if pending_mul[0] is not None:
                        pending_mul[0]()
                        pending_mul[0] = None

                for h in range(HPC):
                    pb, off = h // 2, DK * (h % 2)
                    pv = psB.tile([P, QCW], F32, tag="pv")
                    for g in range(ngr):
                        sc = psA.tile([P, 2 * QCW], F32, tag="sc")
                        rr = []
                        for j in range(2):
                            st = 2 * g + j
                            r = max(P * st - QCW * qc, 0)  # fully-masked cols
                            rr.append(r)
                            nc.tensor.matmul(
                                sc[:, QCW * j + r : QCW * (j + 1)],
                                kt_sb[off : off + DK, pb, P * st : P * (st + 1)],
                                qt_t[off : off + DK, pb, r:QCW],
                                start=True,
                                stop=True,
                            )
                        ex = expp.tile([P, 2 * QCW], BF, tag="ex")
                        # One activation spanning both tiles from the first
                        # unmasked column.  Columns [QCW+.. : QCW+rr[1]] of
                        # tile 1 are unwritten PSUM garbage; exp of them is
                        # finite and never read downstream.
                        nc.scalar.activation(
                            ex[:, rr[0] : 2 * QCW],
                            sc[:, rr[0] : 2 * QCW],
                            EXP,
                            scale=0.125,
                        )
                        for j in range(2):
                            st = 2 * g + j
                            r = P * st - QCW * qc
                            if r >= 0:  # diagonal tile: 0/1 triangle mask
                                reg = ex[:, QCW * j + r : QCW * j + r + P]
                                nc.gpsimd.tensor_tensor(reg, reg, m01_sb[:], MUL)
                        for j in range(2):
                            st = 2 * g + j
                            r = rr[j]
                            nc.tensor.matmul(
                                pv[0:65, r:QCW],
                                v_sb[:, st, h, :],
                                ex[:, QCW * j + r : QCW * (j + 1)],
                                start=(st == 0),
                                stop=(st == nst - 1),
                            )
                        done += 1
                        flush_mul()
                        # On the last chunk, hold a few background units back:
                        # they run during the final normalize chain's DMA
                        # round-trip, keeping the PE busy (and the clock warm)
                        # right before the tail Wo-projection.
                        cap = 3 if qc == NQC - 1 else len(bg)
                        while bg_i < min(cap, len(bg)) and (
                            bg_i * total_groups < done * len(bg)
                        ):
                            emit(bg[bg_i])
                            bg_i += 1
                    # normalize: O / sum(exp).  Reciprocal of the 512 sums is
                    # computed after spreading them across 128 partitions.
                    # The spread DMA issues on the scalar queue right after
                    # the s1 copy (no cross-queue wait); the mul is deferred.
                    s1 = rcpp.tile([1, QCW], F32, tag="s1")
                    nc.scalar.copy(s1[:], pv[64:65, :])
                    s16 = rcpp.tile([16, 32], F32, tag="s16")
                    nc.scalar.dma_start(s16[:], s1[:])
                    r16 = rcpp.tile([16, 32], F32, tag="r16")
                    nc.vector.reciprocal(r16[:], s16[:])
                    rc = rcpp.tile([1, QCW], F32, tag="rc")
                    nc.sync.dma_start(rc[:], r16[:])
                    rb = rbpp.tile([P, QCW], F32, tag="rb")
                    nc.gpsimd.partition_broadcast(rb[:], rc[:])

                    def make_mul(pv=pv, rb=rb, off=off, pb=pb):
                        def m():
                            nc.vector.tensor_tensor(
                                ot_t[off : off + DK, pb, :],
                                pv[0:DK, :],
                                rb[off : off + DK, :],
                                MUL,
                            )
                        return m

                    pending_mul[0] = make_mul()
                while bg_i < len(bg):
                    emit(bg[bg_i])
                    bg_i += 1
                flush_mul()

            # tail: Wo-projection for the last chunk
            for mt in range(KO):
                out_proj(NQC - 1, ots[NQC - 1], mt)

    nc.finalize()
    return nc


_NC_CACHE = []
_LAST_IN_MAPS = []


def _rope_tables(token_positions):
    pos = np.asarray(token_positions).astype(np.float32)
    exponent = np.arange(0, DK, 2, dtype=np.float32)
    inv_freq = (1.0 / (10000.0 ** (exponent / DK))).astype(np.float32)
    freqs = pos[:, None] * inv_freq[None, :]          # [S, 32]
    cos64 = np.repeat(np.cos(freqs).T.astype(np.float32), 2, axis=0)  # [64, S]
    sin64 = np.repeat(np.sin(freqs).T.astype(np.float32), 2, axis=0)
    sgn = np.where(np.arange(DK) % 2 == 0, -1.0, 1.0).astype(np.float32)
    sin64 = sin64 * sgn[:, None]
    cos128 = np.tile(cos64, (2, 1))
    sin128 = np.tile(sin64, (2, 1))
    return cos128.astype(BF_NP), sin128.astype(BF_NP)


def prep_in_maps(x, Wq, Wk, Wv, Wo, token_positions):
    x = np.asarray(x, dtype=np.float32)
    Wq = np.asarray(Wq, dtype=np.float32)
    Wk = np.asarray(Wk, dtype=np.float32)
    Wv = np.asarray(Wv, dtype=np.float32)
    Wo = np.asarray(Wo, dtype=np.float32)
    b = x.shape[0]

    cos128, sin128 = _rope_tables(token_positions)

    psw = np.zeros((P, P), dtype=np.float32)
    idx = np.arange(P)
    psw[idx, idx ^ 1] = 1.0  # swap adjacent pairs
    psw = psw.astype(BF_NP)

    m01 = (np.arange(P)[None, :] >= np.arange(P)[:, None]).astype(BF_NP)

    xts = [np.ascontiguousarray(x[bi].T).astype(BF_NP) for bi in range(b)]

    def warr(w):  # [D, DH] -> [P, KO, DH] with row ko*128+p -> (p, ko)
        return np.ascontiguousarray(
            w.reshape(KO, P, DH).transpose(1, 0, 2)
        ).astype(BF_NP)

    in_maps = []
    cpb = NCORES // b  # cores per batch
    for c in range(NCORES):
        bi, g = c // cpb, c % 4
        cs = slice(DH * g, DH * (g + 1))
        in_maps.append(
            {
                "xt": xts[bi],
                "wq": warr(Wq[:, cs]),
                "wk": warr(Wk[:, cs]),
                "wv": warr(Wv[:, cs]),
                "wo": np.ascontiguousarray(
                    Wo[cs, :].reshape(2, P, D).transpose(1, 0, 2)
                ).astype(BF_NP),
                "cosd": cos128,
                "sind": sin128,
                "pswap": psw,
                "m01d": m01,
            }
        )
    return in_maps


def kernel(x, Wq, Wk, Wv, Wo, token_positions):
    b = np.asarray(x).shape[0]
    in_maps = prep_in_maps(x, Wq, Wk, Wv, Wo, token_positions)

    if not _NC_CACHE:
        _NC_CACHE.append(build_nc())
    nc = _NC_CACHE[0]
    _LAST_IN_MAPS.clear()
    _LAST_IN_MAPS.append(in_maps)

    res = run_bass_kernel_spmd(nc, in_maps, list(range(NCORES)), trace=False)

    y = np.zeros((b, S, D), dtype=np.float32)
    cpb = NCORES // b
    for c in range(NCORES):
        y[c // cpb] += res.results[c]["yt"].T.astype(np.float32)
    return y


# revision 21
# speedup vs baseline: 2.0714x; 2.0714x over previous
"""Causal multi-head attention (RoPE) Trainium2 Bass kernel.

Problem: x[2,2048,1024] @ Wq/Wk/Wv -> 16 heads of causal attention with
interleaved-pair RoPE -> @ Wo.  Sharded over 8 NeuronCores as
(batch x head-group): core c handles batch c//4 and heads [4*(c%4), 4*(c%4)+4).
Each core computes a partial y^T = (attn_out_heads @ Wo[rows]) for its head
group; the host sums the 4 partials per batch and transposes back.

On-device layout is "transposed" throughout: x^T, Q^T, K^T live as
[dims, seq] so every matmul contracts over the partition axis.  Scores are
computed transposed (S^T[kv, q]) so the PV matmul needs no transposes; the
softmax denominator comes from an extra ones-column appended to V.  The
softmax runs without max-subtraction (scores are O(10), exp cannot overflow).

All matmul/DVE operands are bf16 (PSUM accumulation stays fp32): full-rate
PE, fast weight load, half the DMA bytes, 2-4x DVE modes.  The causal mask
is applied as a 0/1 bf16 multiply on exp(scores) (exp of unmasked upper
triangle is finite garbage that the multiply kills).  Emission is software-
pipelined: the projections for chunk qc+1 and the Wo-projection for chunk
qc-1 are interleaved between attention groups of chunk qc, so the PE has
work while the scalar engine computes exp.

RoPE: rope(v) = v*cos + pairswap(v)*sin', where pairswap is a 128x128
permutation matmul and sin' carries the alternating signs.  The reciprocal
of the softmax denominator is computed after spreading the 512 sums across
128 partitions by SBUF-to-SBUF DMA (a [1,512] reciprocal runs on a single
DVE lane and would cost ~4us).
"""

import numpy as np
import ml_dtypes

import concourse.bass as bass
import concourse.mybir as mybir
from concourse import bacc
from concourse.tile import TileContext
from concourse.bass_utils import run_bass_kernel_spmd

P = 128          # partitions
S = 2048         # sequence length
D = 1024         # model dim
DK = 64          # head dim
HPC = 4          # heads per core
DH = HPC * DK    # qkv dims per core (256)
KO = D // P      # 8 contraction slices
QCW = 512        # q chunk width
NQC = S // QCW   # 4 q chunks
NST = S // P     # 16 kv tiles
NCORES = 8

F32 = mybir.dt.float32
BF = mybir.dt.bfloat16
BF_NP = ml_dtypes.bfloat16
EXP = mybir.ActivationFunctionType.Exp
MUL = mybir.AluOpType.mult
ADD = mybir.AluOpType.add


def build_nc():
    nc = bacc.Bacc()

    xt = nc.dram_tensor("xt", [D, S], BF, kind="ExternalInput")
    # weights arrive host-pre-arranged so loads are plain contiguous DMAs:
    # wq[p, ko, m] = Wq[ko*128 + p, m];  wo[p, kb, n] = Wo[kb*128 + p, n]
    wq = nc.dram_tensor("wq", [P, KO, DH], BF, kind="ExternalInput")
    wk = nc.dram_tensor("wk", [P, KO, DH], BF, kind="ExternalInput")
    wv = nc.dram_tensor("wv", [P, KO, DH], BF, kind="ExternalInput")
    wo = nc.dram_tensor("wo", [P, 2, D], BF, kind="ExternalInput")
    cosd = nc.dram_tensor("cosd", [P, S], BF, kind="ExternalInput")
    sind = nc.dram_tensor("sind", [P, S], BF, kind="ExternalInput")
    pswap = nc.dram_tensor("pswap", [P, P], BF, kind="ExternalInput")
    # m01d[p, c] = 1 if c >= p else 0 (keep lower triangle incl. diagonal)
    m01d = nc.dram_tensor("m01d", [P, P], BF, kind="ExternalInput")
    yt = nc.dram_tensor("yt", [D, S], BF, kind="ExternalOutput")

    with TileContext(nc) as tc:
        with (
            tc.tile_pool(name="const", bufs=1) as cp,
            tc.tile_pool(name="qt", bufs=2) as qtp,
            tc.tile_pool(name="ot", bufs=2) as otp,
            tc.tile_pool(name="exps", bufs=3) as expp,
            tc.tile_pool(name="raw", bufs=3) as rawp,
            tc.tile_pool(name="tmp", bufs=3) as tmpp,
            tc.tile_pool(name="rcp", bufs=2) as rcpp,
            tc.tile_pool(name="rbp", bufs=2) as rbpp,
            tc.tile_pool(name="ysb", bufs=3) as ysbp,
            tc.tile_pool(name="psA", bufs=2, space="PSUM") as psA,
            tc.tile_pool(name="psB", bufs=2, space="PSUM") as psB,
            tc.tile_pool(name="psC", bufs=2, space="PSUM") as psC,
        ):
            # ---- constant loads ----
            # gpsimd queue: weights (wk first: it gates the first matmuls);
            # sync/scalar queues: x slices in parallel.
            wk_sb = cp.tile([P, KO, DH], BF, tag="wk")
            nc.gpsimd.dma_start(wk_sb[:, 0:4, :], wk[:, 0:4, :])
            sw_sb = cp.tile([P, P], BF, tag="pswap")
            nc.gpsimd.dma_start(sw_sb[:], pswap[:])
            nc.gpsimd.dma_start(wk_sb[:, 4:KO, :], wk[:, 4:KO, :])
            wv_sb = cp.tile([P, KO, DH], BF, tag="wv")
            nc.gpsimd.dma_start(wv_sb[:], wv[:])
            wq_sb = cp.tile([P, KO, DH], BF, tag="wq")
            nc.gpsimd.dma_start(wq_sb[:], wq[:])
            m01_sb = cp.tile([P, P], BF, tag="m01")
            nc.gpsimd.dma_start(m01_sb[:], m01d[:])
            wo_sb = cp.tile([P, 2, D], BF, tag="wo")
            nc.gpsimd.dma_start(wo_sb[:], wo[:])
            # x arrives in column chunks: the 512 columns chunk 0 needs land
            # before the full rows, so the first projections start early.
            xts = [cp.tile([P, S], BF, tag=f"xt{ko}", name=f"xt{ko}") for ko in range(KO)]
            for ko in range(KO):
                eng = nc.sync if ko % 2 == 0 else nc.scalar
                eng.dma_start(xts[ko][:, 0:QCW], xt[P * ko : P * (ko + 1), 0:QCW])
            cos_sb = cp.tile([P, S], BF, tag="cos")
            nc.sync.dma_start(cos_sb[:, 0:QCW], cosd[:, 0:QCW])
            sin_sb = cp.tile([P, S], BF, tag="sin")
            nc.scalar.dma_start(sin_sb[:, 0:QCW], sind[:, 0:QCW])
            for ko in range(KO):
                eng = nc.sync if ko % 2 == 0 else nc.scalar
                eng.dma_start(xts[ko][:, QCW:S], xt[P * ko : P * (ko + 1), QCW:S])
            nc.sync.dma_start(cos_sb[:, QCW:S], cosd[:, QCW:S])
            nc.scalar.dma_start(sin_sb[:, QCW:S], sind[:, QCW:S])

            # V in [s-rows, dims] layout with a ones column per head:
            # [V(64) | 1] -> PV out rows 0..63 = O^T, row 64 = sum(exp)
            v_sb = cp.tile([P, NST, HPC, DK + 1], BF, tag="v")
            nc.vector.memset(v_sb[:, :, :, DK], 1.0)

            kt_sb = cp.tile([P, 2, S], BF, tag="kt")

            # Warm-up: ~3.4us of back-to-back dummy matmuls on the first
            # tiles to land flips the PE clock-gate to 2.4GHz while the
            # remaining input DMAs are still in flight.
            for _ in range(8):
                wu = psC.tile([P, QCW], F32, tag="proj")
                nc.tensor.matmul(
                    wu[:], sw_sb[:], wk_sb[:, 0:2, :], start=True, stop=True
                )

            def proj_rope(w_sb, dst, pb, qc):
                """dst <- rope( (x @ W)^T )[128 dims block pb, 512 q cols qc]."""
                qs = slice(QCW * qc, QCW * (qc + 1))
                ps = psC.tile([P, QCW], F32, tag="proj")
                for ko in range(KO):
                    nc.tensor.matmul(
                        ps[:],
                        w_sb[:, ko, P * pb : P * (pb + 1)],
                        xts[ko][:, qs],
                        start=(ko == 0),
                        stop=(ko == KO - 1),
                    )
                raw = rawp.tile([P, QCW], BF, tag="raw")
                nc.vector.tensor_copy(raw[:], ps[:])
                swp = psC.tile([P, QCW], F32, tag="proj")
                nc.tensor.matmul(swp[:], sw_sb[:], raw[:], start=True, stop=True)
                nc.vector.tensor_tensor(dst, raw[:], cos_sb[:, qs], MUL)
                tmp = tmpp.tile([P, QCW], BF, tag="tmp")
                nc.vector.tensor_tensor(tmp[:], swp[:], sin_sb[:, qs], MUL)
                nc.vector.tensor_tensor(dst, dst, tmp[:], ADD)

            def v_proj(st):
                ps = psC.tile([P, QCW], F32, tag="proj")
                for ko in range(KO):
                    nc.tensor.matmul(
                        ps[:, :DH],
                        xts[ko][:, P * st : P * (st + 1)],
                        wv_sb[:, ko, :],
                        start=(ko == 0),
                        stop=(ko == KO - 1),
                    )
                vv = ps[:, :DH].rearrange("p (h d) -> p h d", h=HPC)
                nc.vector.tensor_copy(v_sb[:, st, :, 0:DK], vv)

            def out_proj(qc, ot_t, mt):
                """yt[128 rows mt, q chunk qc] = Wo^T[mt] @ O(qc)."""
                qs = slice(QCW * qc, QCW * (qc + 1))
                yp = psC.tile([P, QCW], F32, tag="proj")
                for kb in range(2):
                    nc.tensor.matmul(
                        yp[:],
                        wo_sb[:, kb, P * mt : P * (mt + 1)],
                        ot_t[:, kb, :],
                        start=(kb == 0),
                        stop=(kb == 1),
                    )
                ys = ysbp.tile([P, QCW], BF, tag="ys")
                nc.vector.tensor_copy(ys[:], yp[:])
                nc.sync.dma_start(yt[P * mt : P * (mt + 1), qs], ys[:])

            qts = {}
            ots = {}

            def q_proj(qc, pb):
                if qc not in qts:
                    qts[qc] = qtp.tile([P, 2, QCW], BF, tag="qt", name=f"qt{qc}")
                proj_rope(wq_sb, qts[qc][:, pb, :], pb, qc)

            # ---- prologue: projections for qc 0 ----
            for pb in range(2):
                proj_rope(wk_sb, kt_sb[:, pb, 0:QCW], pb, 0)
            for st in range(4):
                v_proj(st)
            for pb in range(2):
                q_proj(0, pb)

            # ---- software-pipelined main loop ----
            for qc in range(NQC):
                qt_t = qts[qc]

                bg = []  # units interleaved into this qc's attention groups
                if qc >= 1:
                    for mt in range(KO):
                        bg.append(("o", qc - 1, mt))
                if qc + 1 < NQC:
                    for pb in range(2):
                        bg.append(("k", qc + 1, pb))
                    for st in range(4 * (qc + 1), 4 * (qc + 1) + 4):
                        bg.append(("v", st, 0))
                    for pb in range(2):
                        bg.append(("q", qc + 1, pb))

                def emit(u):
                    kind, a, b = u
                    if kind == "o":
                        out_proj(a, ots[a], b)
                    elif kind == "k":
                        proj_rope(wk_sb, kt_sb[:, b, QCW * a : QCW * (a + 1)], b, a)
                    elif kind == "v":
                        v_proj(a)
                    else:
                        q_proj(a, b)

                ot_t = otp.tile([P, 2, QCW], BF, tag="ot")
                ots[qc] = ot_t
                nst = 4 * qc + 4          # kv tiles for this chunk (causal)
                ngr = nst // 2            # processed in pairs
                total_groups = HPC * ngr
                done = 0
                bg_i = 0
                # The normalize multiply of head h is emitted during head
                # h+1's groups so the DVE queue never head-of-line blocks
                # on the partition broadcast.
                pending_mul = [None]

                def flush_mul():
                    if pending_mul[0] is not None:
                        pending_mul[0]()
                        pending_mul[0] = None

                for h in range(HPC):
                    pb, off = h // 2, DK * (h % 2)
                    pv = psB.tile([P, QCW], F32, tag="pv")
                    for g in range(ngr):
                        sc = psA.tile([P, 2 * QCW], F32, tag="sc")
                        rr = []
                        for j in range(2):
                            st = 2 * g + j
                            r = max(P * st - QCW * qc, 0)  # fully-masked cols
                            rr.append(r)
                            nc.tensor.matmul(
                                sc[:, QCW * j + r : QCW * (j + 1)],
                                kt_sb[off : off + DK, pb, P * st : P * (st + 1)],
                                qt_t[off : off + DK, pb, r:QCW],
                                start=True,
                                stop=True,
                            )
                        ex = expp.tile([P, 2 * QCW], BF, tag="ex")
                        # One activation spanning both tiles from the first
                        # unmasked column.  Columns [QCW+.. : QCW+rr[1]] of
                        # tile 1 are unwritten PSUM garbage; exp of them is
                        # finite and never read downstream.
                        nc.scalar.activation(
                            ex[:, rr[0] : 2 * QCW],
                            sc[:, rr[0] : 2 * QCW],
                            EXP,
                            scale=0.125,
                        )
                        for j in range(2):
                            st = 2 * g + j
                            r = P * st - QCW * qc
                            if r >= 0:  # diagonal tile: 0/1 triangle mask
                                reg = ex[:, QCW * j + r : QCW * j + r + P]
                                nc.vector.tensor_tensor(reg, reg, m01_sb[:], MUL)
                        for j in range(2):
                            st = 2 * g + j
                            r = rr[j]
                            nc.tensor.matmul(
                                pv[0:65, r:QCW],
                                v_sb[:, st, h, :],
                                ex[:, QCW * j + r : QCW * (j + 1)],
                                start=(st == 0),
                                stop=(st == nst - 1),
                            )
                        done += 1
                        flush_mul()
                        # On the last chunk, hold a few background units back:
                        # they run during the final normalize chain's DMA
                        # round-trip, keeping the PE busy (and the clock warm)
                        # right before the tail Wo-projection.
                        cap = 3 if qc == NQC - 1 else len(bg)
                        while bg_i < min(cap, len(bg)) and (
                            bg_i * total_groups < done * len(bg)
                        ):
                            emit(bg[bg_i])
                            bg_i += 1
                    # normalize: O / sum(exp).  Reciprocal of the 512 sums is
                    # computed after spreading them across 128 partitions.
                    # The spread DMA issues on the scalar queue right after
                    # the s1 copy (no cross-queue wait); the mul is deferred.
                    s1 = rcpp.tile([1, QCW], F32, tag="s1")
                    nc.scalar.copy(s1[:], pv[64:65, :])
                    s16 = rcpp.tile([16, 32], F32, tag="s16")
                    nc.scalar.dma_start(s16[:], s1[:])
                    r16 = rcpp.tile([16, 32], F32, tag="r16")
                    nc.vector.reciprocal(r16[:], s16[:])
                    rc = rcpp.tile([1, QCW], F32, tag="rc")
                    nc.sync.dma_start(rc[:], r16[:])
                    rb = rbpp.tile([P, QCW], F32, tag="rb")
                    nc.gpsimd.partition_broadcast(rb[:], rc[:])

                    def make_mul(pv=pv, rb=rb, off=off, pb=pb):
                        def m():
                            nc.vector.tensor_tensor(
                                ot_t[off : off + DK, pb, :],
                                pv[0:DK, :],
                                rb[off : off + DK, :],
                                MUL,
                            )
                        return m

                    pending_mul[0] = make_mul()
                while bg_i < len(bg):
                    emit(bg[bg_i])
                    bg_i += 1
                flush_mul()

            # tail: Wo-projection for the last chunk
            for mt in range(KO):
                out_proj(NQC - 1, ots[NQC - 1], mt)

    nc.finalize()
    return nc


_NC_CACHE = []
_LAST_IN_MAPS = []


def _rope_tables(token_positions):
    pos = np.asarray(token_positions).astype(np.float32)
    exponent = np.arange(0, DK, 2, dtype=np.float32)
    inv_freq = (1.0 / (10000.0 ** (exponent / DK))).astype(np.float32)
    freqs = pos[:, None] * inv_freq[None, :]          # [S, 32]
    cos64 = np.repeat(np.cos(freqs).T.astype(np.float32), 2, axis=0)  # [64, S]
    sin64 = np.repeat(np.sin(freqs).T.astype(np.float32), 2, axis=0)
    sgn = np.where(np.arange(DK) % 2 == 0, -1.0, 1.0).astype(np.float32)
    sin64 = sin64 * sgn[:, None]
    cos128 = np.tile(cos64, (2, 1))
    sin128 = np.tile(sin64, (2, 1))
    return cos128.astype(BF_NP), sin128.astype(BF_NP)


def prep_in_maps(x, Wq, Wk, Wv, Wo, token_positions):
    x = np.asarray(x, dtype=np.float32)
    Wq = np.asarray(Wq, dtype=np.float32)
    Wk = np.asarray(Wk, dtype=np.float32)
    Wv = np.asarray(Wv, dtype=np.float32)
    Wo = np.asarray(Wo, dtype=np.float32)
    b = x.shape[0]

    cos128, sin128 = _rope_tables(token_positions)

    psw = np.zeros((P, P), dtype=np.float32)
    idx = np.arange(P)
    psw[idx, idx ^ 1] = 1.0  # swap adjacent pairs
    psw = psw.astype(BF_NP)

    m01 = (np.arange(P)[None, :] >= np.arange(P)[:, None]).astype(BF_NP)

    xts = [np.ascontiguousarray(x[bi].T).astype(BF_NP) for bi in range(b)]

    def warr(w):  # [D, DH] -> [P, KO, DH] with row ko*128+p -> (p, ko)
        return np.ascontiguousarray(
            w.reshape(KO, P, DH).transpose(1, 0, 2)
        ).astype(BF_NP)

    in_maps = []
    cpb = NCORES // b  # cores per batch
    for c in range(NCORES):
        bi, g = c // cpb, c % 4
        cs = slice(DH * g, DH * (g + 1))
        in_maps.append(
            {
                "xt": xts[bi],
                "wq": warr(Wq[:, cs]),
                "wk": warr(Wk[:, cs]),
                "wv": warr(Wv[:, cs]),
                "wo": np.ascontiguousarray(
                    Wo[cs, :].reshape(2, P, D).transpose(1, 0, 2)
                ).astype(BF_NP),
                "cosd": cos128,
                "sind": sin128,
                "pswap": psw,
                "m01d": m01,
            }
        )
    return in_maps


def kernel(x, Wq, Wk, Wv, Wo, token_positions):
    b = np.asarray(x).shape[0]
    in_maps = prep_in_maps(x, Wq, Wk, Wv, Wo, token_positions)

    if not _NC_CACHE:
        _NC_CACHE.append(build_nc())
    nc = _NC_CACHE[0]
    _LAST_IN_MAPS.clear()
    _LAST_IN_MAPS.append(in_maps)

    res = run_bass_kernel_spmd(nc, in_maps, list(range(NCORES)), trace=False)

    y = np.zeros((b, S, D), dtype=np.float32)
    cpb = NCORES // b
    for c in range(NCORES):
        y[c // cpb] += res.results[c]["yt"].T.astype(np.float32)
    return y


# revision 24
# speedup vs baseline: 2.0764x; 1.0024x over previous
"""Causal multi-head attention (RoPE) Trainium2 Bass kernel.

Problem: x[2,2048,1024] @ Wq/Wk/Wv -> 16 heads of causal attention with
interleaved-pair RoPE -> @ Wo.  Sharded over 8 NeuronCores as
(batch x head-group): core c handles batch c//4 and heads [4*(c%4), 4*(c%4)+4).
Each core computes a partial y^T = (attn_out_heads @ Wo[rows]) for its head
group; the host sums the 4 partials per batch and transposes back.

On-device layout is "transposed" throughout: x^T, Q^T, K^T live as
[dims, seq] so every matmul contracts over the partition axis.  Scores are
computed transposed (S^T[kv, q]) so the PV matmul needs no transposes; the
softmax denominator comes from an extra ones-column appended to V.  The
softmax runs without max-subtraction (scores are O(10), exp cannot overflow).

All matmul/DVE operands are bf16 (PSUM accumulation stays fp32): full-rate
PE, fast weight load, half the DMA bytes, 2-4x DVE modes.  The causal mask
is applied as a 0/1 bf16 multiply on exp(scores) (exp of unmasked upper
triangle is finite garbage that the multiply kills).  Emission is software-
pipelined: the projections for chunk qc+1 and the Wo-projection for chunk
qc-1 are interleaved between attention groups of chunk qc, so the PE has
work while the scalar engine computes exp.

RoPE: rope(v) = v*cos + pairswap(v)*sin', where pairswap is a 128x128
permutation matmul and sin' carries the alternating signs.  The reciprocal
of the softmax denominator is computed after spreading the 512 sums across
128 partitions by SBUF-to-SBUF DMA (a [1,512] reciprocal runs on a single
DVE lane and would cost ~4us).
"""

import numpy as np
import ml_dtypes

import concourse.bass as bass
import concourse.mybir as mybir
from concourse import bacc
from concourse.tile import TileContext
from concourse.bass_utils import run_bass_kernel_spmd

P = 128          # partitions
S = 2048         # sequence length
D = 1024         # model dim
DK = 64          # head dim
HPC = 4          # heads per core
DH = HPC * DK    # qkv dims per core (256)
KO = D // P      # 8 contraction slices
QCW = 512        # q chunk width
NQC = S // QCW   # 4 q chunks
NST = S // P     # 16 kv tiles
NCORES = 8

F32 = mybir.dt.float32
BF = mybir.dt.bfloat16
BF_NP = ml_dtypes.bfloat16
EXP = mybir.ActivationFunctionType.Exp
MUL = mybir.AluOpType.mult
ADD = mybir.AluOpType.add


def build_nc():
    nc = bacc.Bacc()

    xt = nc.dram_tensor("xt", [D, S], BF, kind="ExternalInput")
    # weights arrive host-pre-arranged so loads are plain contiguous DMAs:
    # wq[p, ko, m] = Wq[ko*128 + p, m];  wo[p, kb, n] = Wo[kb*128 + p, n]
    wq = nc.dram_tensor("wq", [P, KO, DH], BF, kind="ExternalInput")
    wk = nc.dram_tensor("wk", [P, KO, DH], BF, kind="ExternalInput")
    wv = nc.dram_tensor("wv", [P, KO, DH], BF, kind="ExternalInput")
    wo = nc.dram_tensor("wo", [P, 2, D], BF, kind="ExternalInput")
    cosd = nc.dram_tensor("cosd", [P, S], BF, kind="ExternalInput")
    sind = nc.dram_tensor("sind", [P, S], BF, kind="ExternalInput")
    pswap = nc.dram_tensor("pswap", [P, P], BF, kind="ExternalInput")
    # m01d[p, c] = 1 if c >= p else 0 (keep lower triangle incl. diagonal)
    m01d = nc.dram_tensor("m01d", [P, P], BF, kind="ExternalInput")
    yt = nc.dram_tensor("yt", [D, S], BF, kind="ExternalOutput")

    with TileContext(nc) as tc:
        with (
            tc.tile_pool(name="const", bufs=1) as cp,
            tc.tile_pool(name="qt", bufs=2) as qtp,
            tc.tile_pool(name="ot", bufs=2) as otp,
            tc.tile_pool(name="exps", bufs=3) as expp,
            tc.tile_pool(name="raw", bufs=3) as rawp,
            tc.tile_pool(name="tmp", bufs=3) as tmpp,
            tc.tile_pool(name="rcp", bufs=2) as rcpp,
            tc.tile_pool(name="rbp", bufs=2) as rbpp,
            tc.tile_pool(name="ysb", bufs=3) as ysbp,
            tc.tile_pool(name="psA", bufs=2, space="PSUM") as psA,
            tc.tile_pool(name="psB", bufs=2, space="PSUM") as psB,
            tc.tile_pool(name="psC", bufs=2, space="PSUM") as psC,
        ):
            # ---- constant loads ----
            # gpsimd queue: weights (wk first: it gates the first matmuls);
            # sync/scalar queues: x slices in parallel.
            sw_sb = cp.tile([P, P], BF, tag="pswap")
            nc.gpsimd.dma_start(sw_sb[:], pswap[:])
            wk_sb = cp.tile([P, KO, DH], BF, tag="wk")
            nc.gpsimd.dma_start(wk_sb[:, 0:4, :], wk[:, 0:4, :])
            nc.gpsimd.dma_start(wk_sb[:, 4:KO, :], wk[:, 4:KO, :])
            wv_sb = cp.tile([P, KO, DH], BF, tag="wv")
            nc.gpsimd.dma_start(wv_sb[:], wv[:])
            wq_sb = cp.tile([P, KO, DH], BF, tag="wq")
            nc.gpsimd.dma_start(wq_sb[:], wq[:])
            m01_sb = cp.tile([P, P], BF, tag="m01")
            nc.gpsimd.dma_start(m01_sb[:], m01d[:])
            wo_sb = cp.tile([P, 2, D], BF, tag="wo")
            nc.gpsimd.dma_start(wo_sb[:], wo[:])
            # x arrives in column chunks: the 512 columns chunk 0 needs land
            # before the full rows, so the first projections start early.
            xts = [cp.tile([P, S], BF, tag=f"xt{ko}", name=f"xt{ko}") for ko in range(KO)]
            for ko in range(KO):
                eng = nc.sync if ko % 2 == 0 else nc.scalar
                eng.dma_start(xts[ko][:, 0:QCW], xt[P * ko : P * (ko + 1), 0:QCW])
            cos_sb = cp.tile([P, S], BF, tag="cos")
            nc.sync.dma_start(cos_sb[:, 0:QCW], cosd[:, 0:QCW])
            sin_sb = cp.tile([P, S], BF, tag="sin")
            nc.scalar.dma_start(sin_sb[:, 0:QCW], sind[:, 0:QCW])
            for ko in range(KO):
                eng = nc.sync if ko % 2 == 0 else nc.scalar
                eng.dma_start(xts[ko][:, QCW:S], xt[P * ko : P * (ko + 1), QCW:S])
            nc.sync.dma_start(cos_sb[:, QCW:S], cosd[:, QCW:S])
            nc.scalar.dma_start(sin_sb[:, QCW:S], sind[:, QCW:S])

            # V in [s-rows, dims] layout with a ones column per head:
            # [V(64) | 1] -> PV out rows 0..63 = O^T, row 64 = sum(exp)
            v_sb = cp.tile([P, NST, HPC, DK + 1], BF, tag="v")
            nc.vector.memset(v_sb[:, :, :, DK], 1.0)

            kt_sb = cp.tile([P, 2, S], BF, tag="kt")

            # Warm-up: ~3.4us of back-to-back dummy matmuls on the first tile
            # to land (pairswap, 32KB) flips the PE clock-gate to 2.4GHz
            # while the remaining input DMAs are still in flight.
            for _ in range(34):
                wu = psC.tile([P, QCW], F32, tag="proj")
                nc.tensor.matmul(wu[:, 0:P], sw_sb[:], sw_sb[:], start=True, stop=True)

            def proj_rope(w_sb, dst, pb, qc):
                """dst <- rope( (x @ W)^T )[128 dims block pb, 512 q cols qc]."""
                qs = slice(QCW * qc, QCW * (qc + 1))
                ps = psC.tile([P, QCW], F32, tag="proj")
                for ko in range(KO):
                    nc.tensor.matmul(
                        ps[:],
                        w_sb[:, ko, P * pb : P * (pb + 1)],
                        xts[ko][:, qs],
                        start=(ko == 0),
                        stop=(ko == KO - 1),
                    )
                raw = rawp.tile([P, QCW], BF, tag="raw")
                nc.vector.tensor_copy(raw[:], ps[:])
                swp = psC.tile([P, QCW], F32, tag="proj")
                nc.tensor.matmul(swp[:], sw_sb[:], raw[:], start=True, stop=True)
                nc.vector.tensor_tensor(dst, raw[:], cos_sb[:, qs], MUL)
                tmp = tmpp.tile([P, QCW], BF, tag="tmp")
                nc.vector.tensor_tensor(tmp[:], swp[:], sin_sb[:, qs], MUL)
                nc.vector.tensor_tensor(dst, dst, tmp[:], ADD)

            def v_proj(st):
                ps = psC.tile([P, QCW], F32, tag="proj")
                for ko in range(KO):
                    nc.tensor.matmul(
                        ps[:, :DH],
                        xts[ko][:, P * st : P * (st + 1)],
                        wv_sb[:, ko, :],
                        start=(ko == 0),
                        stop=(ko == KO - 1),
                    )
                vv = ps[:, :DH].rearrange("p (h d) -> p h d", h=HPC)
                nc.vector.tensor_copy(v_sb[:, st, :, 0:DK], vv)

            def out_proj(qc, ot_t, mt):
                """yt[128 rows mt, q chunk qc] = Wo^T[mt] @ O(qc)."""
                qs = slice(QCW * qc, QCW * (qc + 1))
                yp = psC.tile([P, QCW], F32, tag="proj")
                for kb in range(2):
                    nc.tensor.matmul(
                        yp[:],
                        wo_sb[:, kb, P * mt : P * (mt + 1)],
                        ot_t[:, kb, :],
                        start=(kb == 0),
                        stop=(kb == 1),
                    )
                ys = ysbp.tile([P, QCW], BF, tag="ys")
                nc.vector.tensor_copy(ys[:], yp[:])
                nc.sync.dma_start(yt[P * mt : P * (mt + 1), qs], ys[:])

            qts = {}
            ots = {}

            def q_proj(qc, pb):
                if qc not in qts:
                    qts[qc] = qtp.tile([P, 2, QCW], BF, tag="qt", name=f"qt{qc}")
                proj_rope(wq_sb, qts[qc][:, pb, :], pb, qc)

            # ---- prologue: projections for qc 0 ----
            for pb in range(2):
                proj_rope(wk_sb, kt_sb[:, pb, 0:QCW], pb, 0)
            for st in range(4):
                v_proj(st)
            for pb in range(2):
                q_proj(0, pb)

            # ---- software-pipelined main loop ----
            for qc in range(NQC):
                qt_t = qts[qc]

                bg = []  # units interleaved into this qc's attention groups
                if qc >= 1:
                    for mt in range(KO):
                        bg.append(("o", qc - 1, mt))
                if qc + 1 < NQC:
                    for pb in range(2):
                        bg.append(("k", qc + 1, pb))
                    for st in range(4 * (qc + 1), 4 * (qc + 1) + 4):
                        bg.append(("v", st, 0))
                    for pb in range(2):
                        bg.append(("q", qc + 1, pb))

                def emit(u):
                    kind, a, b = u
                    if kind == "o":
                        out_proj(a, ots[a], b)
                    elif kind == "k":
                        proj_rope(wk_sb, kt_sb[:, b, QCW * a : QCW * (a + 1)], b, a)
                    elif kind == "v":
                        v_proj(a)
                    else:
                        q_proj(a, b)

                ot_t = otp.tile([P, 2, QCW], BF, tag="ot")
                ots[qc] = ot_t
                nst = 4 * qc + 4          # kv tiles for this chunk (causal)
                ngr = nst // 2            # processed in pairs
                total_groups = HPC * ngr
                done = 0
                bg_i = 0
                # The normalize multiply of head h is emitted during head
                # h+1's groups so the DVE queue never head-of-line blocks
                # on the partition broadcast.
                pending_mul = [None]

                def flush_mul():
                    if pending_mul[0] is not None:
                        pending_mul[0]()
                        pending_mul[0] = None

                for h in range(HPC):
                    pb, off = h // 2, DK * (h % 2)
                    pv = psB.tile([P, QCW], F32, tag="pv")
                    for g in range(ngr):
                        sc = psA.tile([P, 2 * QCW], F32, tag="sc")
                        rr = []
                        for j in range(2):
                            st = 2 * g + j
                            r = max(P * st - QCW * qc, 0)  # fully-masked cols
                            rr.append(r)
                            nc.tensor.matmul(
                                sc[:, QCW * j + r : QCW * (j + 1)],
                                kt_sb[off : off + DK, pb, P * st : P * (st + 1)],
                                qt_t[off : off + DK, pb, r:QCW],
                                start=True,
                                stop=True,
                            )
                        ex = expp.tile([P, 2 * QCW], BF, tag="ex")
                        # One activation spanning both tiles from the first
                        # unmasked column.  Columns [QCW+.. : QCW+rr[1]] of
                        # tile 1 are unwritten PSUM garbage; exp of them is
                        # finite and never read downstream.
                        nc.scalar.activation(
                            ex[:, rr[0] : 2 * QCW],
                            sc[:, rr[0] : 2 * QCW],
                            EXP,
                            scale=0.125,
                        )
                        for j in range(2):
                            st = 2 * g + j
                            r = P * st - QCW * qc
                            if r >= 0:  # diagonal tile: 0/1 triangle mask
                                reg = ex[:, QCW * j + r : QCW * j + r + P]
                                nc.vector.tensor_tensor(reg, reg, m01_sb[:], MUL)
                        for j in range(2):
                            st = 2 * g + j
                            r = rr[j]
                            nc.tensor.matmul(
                                pv[0:65, r:QCW],
                                v_sb[:, st, h, :],
                                ex[:, QCW * j + r : QCW * (j + 1)],
                                start=(st == 0),
                                stop=(st == nst - 1),
                            )
                        done += 1
                        flush_mul()
                        # On the last chunk, hold a few background units back:
                        # they run during the final normalize chain's DMA
                        # round-trip, keeping the PE busy (and the clock warm)
                        # right before the tail Wo-projection.
                        cap = 3 if qc == NQC - 1 else len(bg)
                        while bg_i < min(cap, len(bg)) and (
                            bg_i * total_groups < done * len(bg)
                        ):
                            emit(bg[bg_i])
                            bg_i += 1
                    # normalize: O / sum(exp).  Reciprocal of the 512 sums is
                    # computed after spreading them across 128 partitions.
                    # The spread DMA issues on the scalar queue right after
                    # the s1 copy (no cross-queue wait); the mul is deferred.
                    s1 = rcpp.tile([1, QCW], F32, tag="s1")
                    nc.scalar.copy(s1[:], pv[64:65, :])
                    s16 = rcpp.tile([16, 32], F32, tag="s16")
                    nc.scalar.dma_start(s16[:], s1[:])
                    r16 = rcpp.tile([16, 32], F32, tag="r16")
                    nc.vector.reciprocal(r16[:], s16[:])
                    rc = rcpp.tile([1, QCW], F32, tag="rc")
                    nc.sync.dma_start(rc[:], r16[:])
                    rb = rbpp.tile([P, QCW], F32, tag="rb")
                    nc.gpsimd.partition_broadcast(rb[:], rc[:])

                    def make_mul(pv=pv, rb=rb, off=off, pb=pb):
                        def m():
                            nc.vector.tensor_tensor(
                                ot_t[off : off + DK, pb, :],
                                pv[0:DK, :],
                                rb[off : off + DK, :],
                                MUL,
                            )
                        return m

                    pending_mul[0] = make_mul()
                while bg_i < len(bg):
                    emit(bg[bg_i])
                    bg_i += 1
                if qc == NQC - 1:
                    # Keep-warm links: a dependency-paced MM/copy chain that
                    # touches the PE every ~700ns through the final normalize
                    # chain so the clock-gate never sees an idle window.
                    link = sw_sb
                    for i in range(7):
                        wps = psC.tile([P, QCW], F32, tag="proj")
                        nc.tensor.matmul(
                            wps[:, 0:P], sw_sb[:], link[:], start=True, stop=True
                        )
                        nl = tmpp.tile([P, P], BF, tag="wl", name=f"wl{i}")
                        nc.vector.tensor_copy(nl[:], wps[:, 0:P])
                        link = nl
                flush_mul()

            # tail: Wo-projection for the last chunk
            for mt in range(KO):
                out_proj(NQC - 1, ots[NQC - 1], mt)

    nc.finalize()
    return nc


_NC_CACHE = []
_LAST_IN_MAPS = []


def _rope_tables(token_positions):
    pos = np.asarray(token_positions).astype(np.float32)
    exponent = np.arange(0, DK, 2, dtype=np.float32)
    inv_freq = (1.0 / (10000.0 ** (exponent / DK))).astype(np.float32)
    freqs = pos[:, None] * inv_freq[None, :]          # [S, 32]
    cos64 = np.repeat(np.cos(freqs).T.astype(np.float32), 2, axis=0)  # [64, S]
    sin64 = np.repeat(np.sin(freqs).T.astype(np.float32), 2, axis=0)
    sgn = np.where(np.arange(DK) % 2 == 0, -1.0, 1.0).astype(np.float32)
    sin64 = sin64 * sgn[:, None]
    cos128 = np.tile(cos64, (2, 1))
    sin128 = np.tile(sin64, (2, 1))
    return cos128.astype(BF_NP), sin128.astype(BF_NP)


def prep_in_maps(x, Wq, Wk, Wv, Wo, token_positions):
    x = np.asarray(x, dtype=np.float32)
    Wq = np.asarray(Wq, dtype=np.float32)
    Wk = np.asarray(Wk, dtype=np.float32)
    Wv = np.asarray(Wv, dtype=np.float32)
    Wo = np.asarray(Wo, dtype=np.float32)
    b = x.shape[0]

    cos128, sin128 = _rope_tables(token_positions)

    psw = np.zeros((P, P), dtype=np.float32)
    idx = np.arange(P)
    psw[idx, idx ^ 1] = 1.0  # swap adjacent pairs
    psw = psw.astype(BF_NP)

    m01 = (np.arange(P)[None, :] >= np.arange(P)[:, None]).astype(BF_NP)

    xts = [np.ascontiguousarray(x[bi].T).astype(BF_NP) for bi in range(b)]

    def warr(w):  # [D, DH] -> [P, KO, DH] with row ko*128+p -> (p, ko)
        return np.ascontiguousarray(
            w.reshape(KO, P, DH).transpose(1, 0, 2)
        ).astype(BF_NP)

    in_maps = []
    cpb = NCORES // b  # cores per batch
    for c in range(NCORES):
        bi, g = c // cpb, c % 4
        cs = slice(DH * g, DH * (g + 1))
        in_maps.append(
            {
                "xt": xts[bi],
                "wq": warr(Wq[:, cs]),
                "wk": warr(Wk[:, cs]),
                "wv": warr(Wv[:, cs]),
                "wo": np.ascontiguousarray(
                    Wo[cs, :].reshape(2, P, D).transpose(1, 0, 2)
                ).astype(BF_NP),
                "cosd": cos128,
                "sind": sin128,
                "pswap": psw,
                "m01d": m01,
            }
        )
    return in_maps


def kernel(x, Wq, Wk, Wv, Wo, token_positions):
    b = np.asarray(x).shape[0]
    in_maps = prep_in_maps(x, Wq, Wk, Wv, Wo, token_positions)

    if not _NC_CACHE:
        _NC_CACHE.append(build_nc())
    nc = _NC_CACHE[0]
    _LAST_IN_MAPS.clear()
    _LAST_IN_MAPS.append(in_maps)

    res = run_bass_kernel_spmd(nc, in_maps, list(range(NCORES)), trace=False)

    y = np.zeros((b, S, D), dtype=np.float32)
    cpb = NCORES // b
    for c in range(NCORES):
        y[c // cpb] += res.results[c]["yt"].T.astype(np.float32)
    return y
